# revision 17
# baseline (speedup 1.0000x reference)
"""GCN (2x GCNConv + global mean pool + FC) on 8 Trainium2 NeuronCores.

Strategy (graph-parallel, dst-partitioned):
  - Nodes are split into 8 contiguous blocks of NPC=12500 (edge-cut partition).
  - GCNConv is linear before the bias, so aggregate the *raw* features first:
        A_norm @ x @ W  ==  (A_norm @ x) @ W
    The gather table for layer 1 is xs = dinv * x (bf16), a kernel input.
  - Self-loops become ordinary edges (src == dst) against the pre-scaled table;
    the remaining dinv[dst] factor is applied after aggregation.
  - Each core processes the edges whose dst lands in its block, grouped by dst
    window (128 dst rows) and by src bucket (32768 rows — dma_gather uses int16
    indices).  Edge messages are fetched with dma_gather (256B rows) and
    scatter-added via one-hot matmuls accumulating in PSUM:
        psum[f, d] += msg[e, f]^T @ onehot[e, d]
  - Between layers, each core's hs = dinv * relu(...) block is AllGather'd so
    every core has the full layer-1 activation table.
  - Global mean pool is folded into a final matmul against Gsel[d, g] =
    dinv[d] * (batch[d] == g); the division by counts, the +b2 bias and the
    final FC run on the host (tiny).
"""

import os
import sys
import time

import numpy as np

for _p in ("/opt/trn_rl_repo", "/root/.axon_site/_ro/trn_rl_repo"):
    if _p not in sys.path and os.path.isdir(_p):
        sys.path.append(_p)

N, E, F_IN, H1, H2, C, G = 100000, 1600000, 128, 128, 64, 10, 64
NCORES = 8

LAST_EXEC_NS = None  # wall-clock of the on-device execution (set per call)
LAST_RESULTS = None
DEBUG_DUMPS = False  # add z/hs/z2/ccout debug output tensors to the program


def _roundup(x, m):
    return (x + m - 1) // m * m


class Plan:
    """Host-side preprocessing: edge slots grouped by (core, dst-window,
    src-bucket).  The slot layout is identical across cores (per-segment
    capacities are maxed over cores), so one SPMD program serves all 8.
    """

    def __init__(self, src, dst, batch, n_nodes, n_graphs, win=128,
                 bucket=32768, group_w=8):
        self.n_nodes = n_nodes
        self.n_graphs = n_graphs
        npc = n_nodes // NCORES
        self.npc = npc
        self.win = win
        self.bucket = bucket
        nw = _roundup(npc, win) // win
        self.nw = nw
        nb = _roundup(n_nodes, bucket) // bucket
        self.nb = nb
        self.bucket_rows = [
            min(bucket, n_nodes - b * bucket) for b in range(nb)
        ]

        deg = np.bincount(dst, minlength=n_nodes).astype(np.float64) + 1.0
        dinv = (1.0 / np.sqrt(deg)).astype(np.float32)
        self.dinv = dinv

        # append self loops
        loops = np.arange(n_nodes, dtype=np.int64)
        s = np.concatenate([src, loops])
        d = np.concatenate([dst, loops])

        core = d // npc
        dloc = d - core * npc
        w = dloc // win
        b = s // bucket

        key = (core * nw + w) * nb + b
        counts = np.bincount(key, minlength=NCORES * nw * nb).reshape(
            NCORES, nw, nb
        )
        cap = _roundup(counts.max(axis=0), 128)  # [nw, nb]; zeros stay zero
        self.cap = cap

        # global slot layout: window-major, bucket segments inside
        flat = cap.reshape(-1)
        g_off = np.zeros(flat.size + 1, dtype=np.int64)
        np.cumsum(flat, out=g_off[1:])
        g_off = g_off[:-1].reshape(nw, nb)
        self.total_slots = int(flat.sum())
        self.n_chunks = self.total_slots // 128

        # per-bucket stream: segments ordered by window
        soff = np.zeros((nw, nb), dtype=np.int64)
        np.cumsum(cap[:-1, :], axis=0, out=soff[1:, :])
        self.stream_len = cap.sum(axis=0)  # [nb]
        self.soff = soff

        # call grouping; keep per-call num_idxs <= 8192 (HW deadlocks with
        # very large in-flight dma_gather calls)
        while True:
            n_groups = _roundup(nw, group_w) // group_w
            grp_num = np.zeros((n_groups, nb), dtype=np.int64)
            for gi in range(n_groups):
                w0, w1 = gi * group_w, min((gi + 1) * group_w, nw)
                grp_num[gi] = cap[w0:w1, :].sum(axis=0)
            if grp_num.max() <= 8192 or group_w == 1:
                break
            group_w //= 2
        self.group_w = group_w
        self.n_groups = n_groups
        self.grp_num = grp_num
        self.grp_soff = np.zeros((n_groups, nb), dtype=np.int64)
        for gi in range(n_groups):
            self.grp_soff[gi] = soff[gi * group_w, :]
        # idx16 column offsets per bucket (in the packed [128, total16] tensor)
        self.boff16 = np.zeros(nb + 1, dtype=np.int64)
        np.cumsum(self.stream_len // 16, out=self.boff16[1:])
        self.total16 = int(self.boff16[-1])

        # per-core packed arrays
        self.idx16 = []   # [128, total16] int16 (16-wrapped, replicated x8)
        self.dstloc_packed = []  # [128, n_chunks] f32 (cast later)
        self.batchloc = []
        self.dinvw = []
        for c in range(NCORES):
            sel = np.flatnonzero(core == c)
            ww = w[sel]
            bb = b[sel]
            seg = ww * nb + bb
            order = np.argsort(seg, kind="stable")
            sel = sel[order]
            ww = ww[order]
            bb = bb[order]
            seg = seg[order]
            cnt = np.bincount(seg, minlength=nw * nb)
            start = np.zeros(nw * nb, dtype=np.int64)
            np.cumsum(cnt[:-1], out=start[1:])
            pos = np.arange(sel.size) - start[seg]

            gslot = g_off[ww, bb] + pos
            dl_slot = np.full(self.total_slots, 255.0, dtype=np.float32)
            dl_slot[gslot] = (dloc[sel] % win).astype(np.float32)
            self.dstloc_packed.append(
                np.ascontiguousarray(dl_slot.reshape(-1, 128).T)
            )

            # per-bucket idx streams (window-major), 16-wrapped
            ix = np.zeros(int(self.stream_len.sum()), dtype=np.int16)
            stream_base = np.zeros(nb, dtype=np.int64)
            np.cumsum(self.stream_len[:-1], out=stream_base[1:])
            sslot = stream_base[bb] + soff[ww, bb] + pos
            ix[sslot] = (s[sel] - bb * bucket).astype(np.int16)
            wrapped = np.zeros((16, self.total16), dtype=np.int16)
            for bi in range(nb):
                seg_ix = ix[stream_base[bi]:stream_base[bi]
                            + self.stream_len[bi]]
                wrapped[:, self.boff16[bi]:self.boff16[bi + 1]] = (
                    seg_ix.reshape(-1, 16).T
                )
            self.idx16.append(np.ascontiguousarray(np.tile(wrapped, (8, 1))))

            node_rows = np.arange(nw * win)
            valid = node_rows < npc
            gnode = np.minimum(c * npc + node_rows, n_nodes - 1)
            bl = np.where(valid, batch[gnode].astype(np.float32), 255.0)
            dv = np.where(valid, dinv[gnode], 0.0).astype(np.float32)
            self.batchloc.append(np.ascontiguousarray(bl.reshape(nw, win).T))
            self.dinvw.append(np.ascontiguousarray(dv.reshape(nw, win).T))

        self.counts_per_graph = np.bincount(batch, minlength=n_graphs).astype(
            np.float32
        )


def _build_bass(plan, f_in, h1, h2):
    from contextlib import ExitStack

    from concourse import bacc, bass, mybir, tile
    from concourse.masks import make_identity

    bf16 = mybir.dt.bfloat16
    f32 = mybir.dt.float32
    i16 = mybir.dt.int16

    n_nodes = plan.n_nodes
    npc = plan.npc
    nw = plan.nw
    nb = plan.nb
    nch = plan.n_chunks
    ng = plan.n_graphs
    bucket = plan.bucket

    nc = bacc.Bacc("TRN2", debug=False, target_bir_lowering=False,
                   num_devices=NCORES)

    # one consolidated bf16 const tensor => one DMA => one semaphore lane
    ncc = 128 + h1 + h1 + h2 + nch + nw + nw
    xs_d = nc.dram_tensor("xs", [n_nodes, f_in], bf16, kind="ExternalInput")
    idx_d = nc.dram_tensor("idx", [128, plan.total16], i16,
                           kind="ExternalInput")
    consts_d = nc.dram_tensor("consts", [128, ncc], bf16, kind="ExternalInput")
    out_d = nc.dram_tensor("out", [ng, h2], f32, kind="ExternalOutput")
    if DEBUG_DUMPS:
        dbg_z = nc.dram_tensor("dbg_z", [128, nw * 128], bf16,
                               kind="ExternalOutput")
        dbg_hs = nc.dram_tensor("dbg_hs", [128, nw * h1], bf16,
                                kind="ExternalOutput")
        dbg_z2 = nc.dram_tensor("dbg_z2", [128, nw * 128], bf16,
                                kind="ExternalOutput")
        dbg_cc = nc.dram_tensor("dbg_cc", [n_nodes, h1], bf16,
                                kind="ExternalOutput")

    cc_in = nc.dram_tensor("cc_in", [npc, h1], bf16)
    cc_out = nc.dram_tensor("cc_out", [n_nodes, h1], bf16, addr_space="Shared")

    max_cols = [
        max(1, int(plan.grp_num[:, b].max()) // 128) for b in range(nb)
    ]
    max_cols16 = [max(8, int(plan.grp_num[:, b].max()) // 16)
                  for b in range(nb)]

    with tile.TileContext(nc) as tc, ExitStack() as es:
        cpool = es.enter_context(tc.tile_pool(name="const", bufs=1))
        gpool = es.enter_context(tc.tile_pool(name="gather", bufs=2))
        ixpool = es.enter_context(tc.tile_pool(name="ix", bufs=2))
        ohpool = es.enter_context(tc.tile_pool(name="onehot", bufs=4))
        bigpool = es.enter_context(tc.tile_pool(name="big", bufs=1))
        pz_pool = es.enter_context(tc.tile_pool(name="pz", bufs=3, space="PSUM"))
        pu_pool = es.enter_context(tc.tile_pool(name="pu", bufs=2, space="PSUM"))
        pt_pool = es.enter_context(tc.tile_pool(name="pt", bufs=2, space="PSUM"))
        ps_pool = es.enter_context(tc.tile_pool(name="ps", bufs=1, space="PSUM"))
        spool = es.enter_context(tc.tile_pool(name="small", bufs=2))

        # ---- constants ----
        ident_sb = cpool.tile([128, 128], bf16, tag="ident")
        make_identity(nc, ident_sb[:])
        consts_sb = cpool.tile([128, ncc], bf16, tag="consts")
        nc.sync.dma_start(out=consts_sb[:], in_=consts_d[:, :])
        o = 0
        iota_sb = consts_sb[:, o:o + 128]; o += 128
        b1b_sb = consts_sb[:, o:o + h1]; o += h1
        w1_sb = consts_sb[:, o:o + h1]; o += h1
        w2_sb = consts_sb[:, o:o + h2]; o += h2
        dstloc_sb = consts_sb[:, o:o + nch]; o += nch
        batchloc_sb = consts_sb[:, o:o + nw]; o += nw
        dinvw_sb = consts_sb[:, o:o + nw]; o += nw

        def scatter_layer(table_dram, z_sb):
            """z_sb[f, w*128+d] = sum_e msg[e, f] * onehot[e, d] (unscaled)."""
            tiles = {}

            def issue(gi):
                for b in range(nb):
                    num = int(plan.grp_num[gi, b])
                    if num == 0:
                        continue
                    c16 = num // 16
                    o16 = int(plan.boff16[b] + plan.grp_soff[gi, b] // 16)
                    it = ixpool.tile([128, max_cols16[b]], i16, tag=f"ix{b}")
                    nc.sync.dma_start(
                        out=it[:, :c16], in_=idx_d[:, o16:o16 + c16]
                    )
                    g = gpool.tile([128, max_cols[b], f_in], bf16,
                                   tag=f"g{b}")
                    nc.gpsimd.dma_gather(
                        out_ap=g[:, :num // 128, :],
                        in_ap=table_dram[
                            b * bucket:b * bucket + plan.bucket_rows[b], :
                        ],
                        idxs_ap=it[:, :c16],
                        num_idxs=num,
                        num_idxs_reg=num,
                        elem_size=f_in,
                        single_packet=False,
                    )
                    tiles[(gi, b)] = g

            issue(0)
            ch = 0
            for gi in range(plan.n_groups):
                if gi + 1 < plan.n_groups:
                    issue(gi + 1)
                w0 = gi * plan.group_w
                w1 = min(w0 + plan.group_w, nw)
                for w in range(w0, w1):
                    nchw = int(plan.cap[w].sum()) // 128
                    if nchw == 0:
                        continue
                    psum_z = pz_pool.tile([128, 128], f32, space="PSUM",
                                          tag="pz")
                    k = 0
                    for b in range(nb):
                        segch = int(plan.cap[w, b]) // 128
                        if segch == 0:
                            continue
                        g = tiles[(gi, b)]
                        cc0 = int(
                            (plan.soff[w, b] - plan.grp_soff[gi, b]) // 128
                        )
                        for kk in range(segch):
                            col = ch + k
                            oh = ohpool.tile([128, 128], bf16, tag="oh")
                            nc.vector.tensor_tensor(
                                out=oh[:],
                                in0=iota_sb[:],
                                in1=dstloc_sb[:, col:col + 1].to_broadcast(
                                    [128, 128]
                                ),
                                op=mybir.AluOpType.is_equal,
                            )
                            nc.tensor.matmul(
                                out=psum_z[:],
                                lhsT=g[:, cc0 + kk, :],
                                rhs=oh[:],
                                start=(k == 0),
                                stop=(k == nchw - 1),
                            )
                            k += 1
                    nc.vector.tensor_copy(
                        out=z_sb[:, w * 128:(w + 1) * 128], in_=psum_z[:]
                    )
                    ch += nchw

        def dense_phase(z_sb, w_sb, hh, u_sb):
            """u_sb[hh, :] = (w_sb^T @ z_sb), bf16 copy out of PSUM."""
            total = nw * 128
            step = 512
            for c0 in range(0, total, step):
                cq = min(step, total - c0)
                psum_u = pu_pool.tile([128, step], f32, space="PSUM", tag="pu")
                nc.tensor.matmul(
                    out=psum_u[:hh, :cq],
                    lhsT=w_sb[:],
                    rhs=z_sb[:, c0:c0 + cq],
                    start=True,
                    stop=True,
                )
                nc.vector.tensor_copy(
                    out=u_sb[:hh, c0:c0 + cq], in_=psum_u[:hh, :cq]
                )

        # ================= layer 1 =================
        z_sb = bigpool.tile([128, nw * 128], bf16, tag="z")
        scatter_layer(xs_d, z_sb)
        if DEBUG_DUMPS:
            nc.sync.dma_start(out=dbg_z[:, :], in_=z_sb[:])

        u_sb = bigpool.tile([128, nw * 128], bf16, tag="u")
        dense_phase(z_sb, w1_sb, h1, u_sb)

        hs_sb = bigpool.tile([128, nw, h1], bf16, tag="z")
        for w in range(nw):
            psum_t = pt_pool.tile([128, 128], bf16, space="PSUM", tag="pt")
            nc.tensor.transpose(
                out=psum_t[:],
                in_=u_sb[:, w * 128:(w + 1) * 128],
                identity=ident_sb[:],
            )
            t1 = spool.tile([128, h1], bf16, tag="t1")
            nc.vector.tensor_tensor(
                out=t1[:],
                in0=psum_t[:],
                in1=dinvw_sb[:, w:w + 1].to_broadcast([128, h1]),
                op=mybir.AluOpType.mult,
            )
            t2 = spool.tile([128, h1], bf16, tag="t2")
            nc.vector.tensor_tensor(
                out=t2[:], in0=t1[:], in1=b1b_sb[:], op=mybir.AluOpType.add
            )
            t3 = spool.tile([128, h1], bf16, tag="t3")
            nc.vector.tensor_scalar_max(out=t3[:], in0=t2[:], scalar1=0.0)
            nc.vector.tensor_tensor(
                out=hs_sb[:, w, :],
                in0=t3[:],
                in1=dinvw_sb[:, w:w + 1].to_broadcast([128, h1]),
                op=mybir.AluOpType.mult,
            )

        # node-major DRAM write: cc_in[w*128 + p, f] = hs_sb[p, w, f]
        cc_in_ap = cc_in[: (npc // 128) * 128, :].rearrange(
            "(w p) f -> p w f", p=128
        )
        nc.sync.dma_start(out=cc_in_ap, in_=hs_sb[:, :npc // 128, :])
        if npc % 128:
            w0 = npc // 128
            nc.sync.dma_start(
                out=cc_in[w0 * 128:npc, :],
                in_=hs_sb[:npc % 128, w0, :],
            )

        nc.gpsimd.collective_compute(
            "AllGather",
            mybir.AluOpType.bypass,
            replica_groups=[list(range(NCORES))],
            ins=[cc_in[:, :].opt()],
            outs=[cc_out[:, :].opt()],
        )
        # absorb the collective-completion wait on the Pool engine so the L2
        # gathers (which also carry a slot-WAR wait) never need 2 waits
        cc_probe = spool.tile([1, h1], bf16, tag="ccprobe")
        nc.gpsimd.dma_start(out=cc_probe[:], in_=cc_out[0:1, :])
        if DEBUG_DUMPS:
            nc.sync.dma_start(out=dbg_hs[:, :],
                              in_=hs_sb[:].rearrange("p w f -> p (w f)"))
            nc.gpsimd.dma_start(out=dbg_cc[:, :], in_=cc_out[:, :])

        # ================= layer 2 =================
        z2_sb = bigpool.tile([128, nw * 128], bf16, tag="z")
        scatter_layer(cc_out, z2_sb)
        if DEBUG_DUMPS:
            nc.sync.dma_start(out=dbg_z2[:, :], in_=z2_sb[:])

        u2_sb = bigpool.tile([128, nw * 128], bf16, tag="u")
        dense_phase(z2_sb, w2_sb, h2, u2_sb)

        # pool: psum_S[g, hf] += Gsel[d, g]^T @ h2_nm[d, hf]
        psum_s = ps_pool.tile([128, 128], f32, space="PSUM", tag="ps")
        for w in range(nw):
            psum_t2 = pt_pool.tile([128, h2], bf16, space="PSUM", tag="pt")
            nc.tensor.transpose(
                out=psum_t2[:],
                in_=u2_sb[:h2, w * 128:(w + 1) * 128],
                identity=ident_sb[:h2, :h2],
            )
            h2nm = spool.tile([128, h2], bf16, tag="h2nm")
            nc.vector.tensor_copy(out=h2nm[:], in_=psum_t2[:])
            gs0 = spool.tile([128, ng], bf16, tag="gs0")
            nc.vector.tensor_tensor(
                out=gs0[:],
                in0=iota_sb[:, :ng],
                in1=batchloc_sb[:, w:w + 1].to_broadcast([128, ng]),
                op=mybir.AluOpType.is_equal,
            )
            gsel = spool.tile([128, ng], bf16, tag="gsel")
            nc.vector.tensor_tensor(
                out=gsel[:],
                in0=gs0[:],
                in1=dinvw_sb[:, w:w + 1].to_broadcast([128, ng]),
                op=mybir.AluOpType.mult,
            )
            nc.tensor.matmul(
                out=psum_s[:ng, :h2],
                lhsT=gsel[:],
                rhs=h2nm[:],
                start=(w == 0),
                stop=(w == nw - 1),
            )
        s_sb = spool.tile([ng, h2], f32, tag="s")
        nc.vector.tensor_copy(out=s_sb[:], in_=psum_s[:ng, :h2])
        nc.sync.dma_start(out=out_d[:, :], in_=s_sb[:])

    nc.compile()
    return nc


def pack_consts(plan, c, W1, W2, b1, h1, h2):
    from concourse import mybir
    bf16_np = mybir.dt.np(mybir.dt.bfloat16)
    iota = np.broadcast_to(np.arange(128, dtype=np.float32), (128, 128))
    b1b = np.broadcast_to(b1, (128, h1))
    parts = [
        iota,
        b1b,
        W1,
        W2,
        plan.dstloc_packed[c],
        plan.batchloc[c],
        plan.dinvw[c],
    ]
    return np.ascontiguousarray(
        np.concatenate([np.asarray(p, dtype=np.float32) for p in parts], axis=1)
    ).astype(bf16_np)


_CACHE = {}


def _get_program(plan, f_in, h1, h2):
    key = (plan.n_nodes, plan.total16, plan.n_chunks,
           tuple(plan.cap.reshape(-1).tolist()))
    if key not in _CACHE:
        _CACHE[key] = _build_bass(plan, f_in, h1, h2)
    return _CACHE[key]


def kernel(x, edge_index, batch, W1, b1, W2, b2, Wfc, bfc):
    global LAST_EXEC_NS, LAST_RESULTS
    from concourse import bass_utils, mybir

    bf16_np = mybir.dt.np(mybir.dt.bfloat16)

    x = np.asarray(x, dtype=np.float32)
    W1 = np.asarray(W1, dtype=np.float32)
    b1 = np.asarray(b1, dtype=np.float32)
    W2 = np.asarray(W2, dtype=np.float32)
    b2 = np.asarray(b2, dtype=np.float32)
    Wfc = np.asarray(Wfc, dtype=np.float32)
    bfc = np.asarray(bfc, dtype=np.float32)
    ei = np.asarray(edge_index)
    src = ei[0].astype(np.int64)
    dst = ei[1].astype(np.int64)
    bat = np.asarray(batch).astype(np.int64)
    n, f_in = x.shape
    h1 = W1.shape[1]
    h2 = W2.shape[1]

    plan = Plan(src, dst, bat, n, G)
    nc = _get_program(plan, f_in, h1, h2)

    xs = (x * plan.dinv[:, None]).astype(bf16_np)

    in_maps = []
    for c in range(NCORES):
        in_maps.append({
            "xs": xs,
            "idx": plan.idx16[c],
            "consts": pack_consts(plan, c, W1, W2, b1, h1, h2),
        })

    t0 = time.perf_counter()
    res = bass_utils.run_bass_kernel_spmd(
        nc, in_maps, core_ids=list(range(NCORES))
    )
    t1 = time.perf_counter()
    LAST_EXEC_NS = int((t1 - t0) * 1e9)
    LAST_RESULTS = res

    # host epilogue: sum partials, mean, +b2, FC
    S = np.zeros((G, h2), dtype=np.float64)
    for c in range(NCORES):
        S += np.asarray(res.results[c]["out"], dtype=np.float64)
    cnt = plan.counts_per_graph.astype(np.float64)
    mean = np.where(
        cnt[:, None] > 0, S / np.maximum(cnt, 1.0)[:, None] + b2[None, :], 0.0
    )
    out = mean.astype(np.float32) @ Wfc + bfc[None, :]
    return out.astype(np.float32)
